# revision 31
# baseline (speedup 1.0000x reference)
"""AdaPT_Linear (per-tensor int8-quantized linear) on 8 trn2 NeuronCores.

Strategy (data-parallel over rows of x), v7 — collective-free, compact I/O:
  - Host passes x.T shards [1024, 2048] and w.T [1024, 1024] (replicated)
    as f16, so SBUF loads land with the contraction (IN) axis on
    partitions, no on-device transposes, and half the input bytes. The
    quantization scales stay the exact f32 ones (computed host-side from
    the f32 tensors), so the only deviation from the reference is the
    rare rounding flip where the f16-rounded value crosses a
    round-to-int boundary (measured ~3e-3 rel err vs the 2e-2 gate).
  - The per-tensor scales are O(1) scalars (127/absmax). v1 computed
    them on-device via an AllGather whose barrier + mesh trigger latency
    (~64us dead time on this runtime) dominated the kernel; they are now
    host-computed during input sharding and shipped in a [128, 4] consts
    tensor. Every O(N) stage - quantize x, quantize w, matmul, dequant,
    bias - runs on device.
  - Quantized int8 values are exact in bf16; int8xint8 products accumulate
    exactly in fp32 PSUM (max |acc| = 127*127*1024 < 2^24), so the bf16
    matmul reproduces the reference int32 matmul bit-exactly.
  - round-half-even matches jnp.round via the +/-1.5*2^23 magic constant;
    clip is a no-op because the scale uses the global abs-max.
  - Output is stored bf16 (host upcasts to f32): halves store traffic.
  - Engine discipline (v2-v6 lessons): a dma_start occupies the issuing
    engine's queue slot until the transfer drains, so ALL DMAs go on the
    otherwise-idle sync engine, in consumption order. The ACT engine runs
    quantize pass-1 for x (scale*v + magic -> f32 scratch); GpSimd runs
    pass-1 for the w n0-halves (f32 output is its fast path - its bf16
    dtype-cast ucode is ~20x slow and stalls concurrent DVE ops) so ACT
    alone does not pace block (0,0), plus the bias partition_broadcast.
    The DVE runs the bias chain, every pass-2 downcast and every PSUM
    dequant (fused mult+bias STT), emitted in consumption order so late
    x-half1 work can't head-of-line block a block-(0,*) dequant.
  - Matmul: row-half x outcol-half blocks of 8 PSUM banks, k-outer
    inside each block so the PE consumes (x,w) k-tile pairs in delivery
    order; the final block instead runs k-INNER per bank, so each bank's
    dequant+store overlaps the next bank's matmuls and the post-last-
    matmul tail is a single STT + store.
"""
import numpy as np

import concourse.bacc as bacc
import concourse.mybir as mybir
import concourse.tile as tile
from concourse import library_config
from concourse.bass_utils import run_bass_kernel_spmd

N_CORES = 8
N_ROWS = 16384
SIZE_IN = 1024
SIZE_OUT = 1024
ROWS_PER_CORE = N_ROWS // N_CORES          # 2048
K_TILES = SIZE_IN // 128                   # 8
R_HALF = ROWS_PER_CORE // 2                # 1024 rows per half
MAGIC = 12582912.0                         # 1.5 * 2**23: round-half-even trick
MAXV = 127.0

F32 = mybir.dt.float32
F16 = mybir.dt.float16
BF16 = mybir.dt.bfloat16


def build_nc():
    nc = bacc.Bacc(None, target_bir_lowering=False, debug=False,
                   num_devices=N_CORES)

    xt_ext = nc.declare_dram_parameter("xt", [SIZE_IN, ROWS_PER_CORE], F16,
                                       isOutput=False)
    wt_ext = nc.declare_dram_parameter("wt", [SIZE_IN, SIZE_OUT], F16,
                                       isOutput=False)
    b_ext = nc.declare_dram_parameter("bias", [1, SIZE_OUT], F32,
                                      isOutput=False)
    c_ext = nc.declare_dram_parameter("consts", [128, 4], F32,
                                      isOutput=False)
    out_ext = nc.declare_dram_parameter("out", [ROWS_PER_CORE, SIZE_OUT],
                                        BF16, isOutput=True)

    with tile.TileContext(nc) as tc:
        with (
            tc.tile_pool(name="big", bufs=1) as big,
            tc.tile_pool(name="scr", bufs=6) as scr,
            tc.tile_pool(name="stats", bufs=1) as stats,
            tc.tile_pool(name="ostage", bufs=8) as ostage,
            tc.tile_pool(name="psum", bufs=8, space="PSUM") as psum_pool,
        ):
            xt_sb = [big.tile([128, ROWS_PER_CORE], F16, tag=f"xt{k}",
                              name=f"xt{k}") for k in range(K_TILES)]
            qxt_sb = [big.tile([128, ROWS_PER_CORE], BF16, tag=f"qxt{k}",
                               name=f"qxt{k}") for k in range(K_TILES)]
            wt_sb = [big.tile([128, SIZE_OUT], F16, tag=f"wt{k}",
                              name=f"wt{k}") for k in range(K_TILES)]
            qwt_sb = [big.tile([128, SIZE_OUT], BF16, tag=f"qwt{k}",
                               name=f"qwt{k}") for k in range(K_TILES)]

            consts = stats.tile([128, 4], F32, tag="consts")
            b_sb = stats.tile([1, SIZE_OUT], F32, tag="b_sb")
            bmax = stats.tile([1, 1], F32, tag="bmax")
            rb = stats.tile([1, 1], F32, tag="rb")
            sb_t = stats.tile([1, 1], F32, tag="sb")
            bq = stats.tile([1, SIZE_OUT], F32, tag="bq")
            bval = stats.tile([1, SIZE_OUT], F32, tag="bval")
            bias_full = stats.tile([128, SIZE_OUT], F32, tag="bias_full")

            sa = consts[:, 0:1]
            sw = consts[:, 1:2]
            sd = consts[:, 2:3]
            magic_c = consts[:, 3:4]

            # gpsimd ucode library for partition_broadcast (bias row)
            nc.gpsimd.load_library(library_config.attn)

            # ---- PE pre-warm: the HAM clock gate needs ~3.4us of
            #      activity before granting 2.4GHz and re-throttles on a
            #      >~2us idle gap. 24 throwaway matmuls on memset scratch
            #      run during the DMA ramp, ending within ~1us of the
            #      first real matmul; with the k1-k3 inputs half-tiled
            #      (below) every later gap stays under ~1.5us, so the
            #      stream runs warm end to end. ----
            warm_in = stats.tile([128, 640], BF16, tag="warm_in")
            warm_ps = psum_pool.tile([128, 512], F32, tag="ps",
                                     name="warm_ps")
            nc.vector.memset(warm_in[:], 0.0)
            for _ in range(24):
                nc.tensor.matmul(warm_ps[:], warm_in[:, 0:128],
                                 warm_in[:, 128:640], start=True, stop=True)

            h0 = slice(0, R_HALF)
            h1 = slice(R_HALF, ROWS_PER_CORE)
            n0 = slice(0, 512)
            n1 = slice(512, SIZE_OUT)
            q0 = slice(0, 512)
            q1 = slice(512, R_HALF)

            # ---- all DMAs on the sync queue, in consumption order.
            #      k0 is split into 512-wide chunks so the first matmul's
            #      inputs clear the DMA-rate ramp as early as possible;
            #      w n1-halves defer until after block (0,0)'s inputs. ----
            nc.sync.dma_start(consts[:], c_ext[:])
            nc.sync.dma_start(b_sb[:], b_ext[:])
            nc.sync.dma_start(xt_sb[0][:, q0], xt_ext[0:128, q0])
            nc.sync.dma_start(wt_sb[0][:, n0], wt_ext[0:128, n0])
            nc.sync.dma_start(xt_sb[0][:, q1], xt_ext[0:128, q1])
            nc.sync.dma_start(wt_sb[0][:, n1], wt_ext[0:128, n1])
            # k1-k3 arrive during the DMA-rate ramp: half-tile them so
            # each k-step's leading data (x q0 + w n0) is 256KB, keeping
            # the PE's inter-k gaps short enough to hold the warm clock
            for k in range(1, 4):
                ks = slice(k * 128, (k + 1) * 128)
                nc.sync.dma_start(xt_sb[k][:, q0], xt_ext[ks, q0])
                nc.sync.dma_start(wt_sb[k][:, n0], wt_ext[ks, n0])
                nc.sync.dma_start(xt_sb[k][:, q1], xt_ext[ks, q1])
                nc.sync.dma_start(wt_sb[k][:, n1], wt_ext[ks, n1])
            for k in range(4, K_TILES):
                nc.sync.dma_start(xt_sb[k][:, h0],
                                  xt_ext[k * 128:(k + 1) * 128, h0])
                nc.sync.dma_start(wt_sb[k][:],
                                  wt_ext[k * 128:(k + 1) * 128, :])
            for k in range(K_TILES):
                nc.sync.dma_start(xt_sb[k][:, h1],
                                  xt_ext[k * 128:(k + 1) * 128, h1])

            # ---- bias chain first on DVE (inputs land immediately;
            #      keeps it clear of the quantize/dequant stream) ----
            nc.vector.tensor_reduce(
                bmax[:], b_sb[:], axis=mybir.AxisListType.X,
                op=mybir.AluOpType.max, apply_absolute_value=True)
            nc.vector.reciprocal(rb[:], bmax[:])
            nc.vector.tensor_scalar_mul(sb_t[:], rb[:], MAXV)
            nc.vector.tensor_scalar(
                bq[:], b_sb[:], sb_t[:], MAGIC,
                op0=mybir.AluOpType.mult, op1=mybir.AluOpType.add)
            nc.vector.tensor_scalar(
                bq[:], bq[:], -MAGIC, None, op0=mybir.AluOpType.add,
                op1=mybir.AluOpType.bypass)
            # bias value row = qb/sb = qb * bmax / 127
            nc.vector.tensor_scalar(
                bval[:], bq[:], bmax[:], 1.0 / MAXV,
                op0=mybir.AluOpType.mult, op1=mybir.AluOpType.mult)
            # (bias_full broadcast is emitted after the w-n0 quantize so
            # it cannot block the gpsimd w pass-1 ops; it is only needed
            # at the first dequant ~30us in)

            # ---- quantize: ACT does scale*v + magic into an f32 scratch
            #      (round-half-even to integer via the magic add), DVE
            #      subtracts magic and downcasts to bf16 (ints <= 127 are
            #      exact in bf16). Block (0,0)'s needs come first: x half0
            #      and only the n0 halves of w. ----
            def quant(src, dst, scale, cols, p1_engine=None):
                t = scr.tile([128, cols], F32, tag="scr")
                if p1_engine is None:
                    nc.scalar.activation(
                        t[:], src, mybir.ActivationFunctionType.Identity,
                        bias=magic_c, scale=scale)
                else:
                    # gpsimd pass-1: f32 output only (its dtype-cast
                    # ucode is slow, but f32 math is fast)
                    p1_engine.tensor_scalar(
                        t[:], src, scale, MAGIC,
                        op0=mybir.AluOpType.mult, op1=mybir.AluOpType.add)
                nc.vector.tensor_scalar(
                    dst, t[:], -MAGIC, None,
                    op0=mybir.AluOpType.add, op1=mybir.AluOpType.bypass)

            # w n0-halves quantize pass-1 on GpSimd so the ACT engine
            # (x pass-1) alone no longer paces block (0,0)
            quant(xt_sb[0][:, q0], qxt_sb[0][:, q0], sa, 512)
            quant(wt_sb[0][:, n0], qwt_sb[0][:, n0], sw, 512,
                  p1_engine=nc.gpsimd)
            quant(xt_sb[0][:, q1], qxt_sb[0][:, q1], sa, 512)
            for k in range(1, 4):
                quant(xt_sb[k][:, q0], qxt_sb[k][:, q0], sa, 512)
                quant(wt_sb[k][:, n0], qwt_sb[k][:, n0], sw, 512,
                      p1_engine=nc.gpsimd)
                quant(xt_sb[k][:, q1], qxt_sb[k][:, q1], sa, 512)
            for k in range(4, K_TILES):
                quant(xt_sb[k][:, h0], qxt_sb[k][:, h0], sa, R_HALF)
                quant(wt_sb[k][:, n0], qwt_sb[k][:, n0], sw, 512,
                      p1_engine=nc.gpsimd)
            # n1 halves of w: needed only from block (0,1) onward
            quant(wt_sb[0][:, n1], qwt_sb[0][:, n1], sw, 512)
            for k in range(1, K_TILES):
                quant(wt_sb[k][:, n1], qwt_sb[k][:, n1], sw, 512)

            nc.gpsimd.partition_broadcast(bias_full[:], bval[:], channels=128)

            # ---- matmul: blocks of PSUM banks, k-outer inside each block
            #      so the PE consumes (x,w) k-tiles in delivery order.
            #      Dequant (fused mult+bias-add STT on DVE) fires per-bank
            #      right after its k=7 matmul. ----
            def mm_block(half, n, r_lo, r_hi):
                ps = {r: psum_pool.tile([128, 512], F32, tag="ps",
                                        name=f"ps_h{half}n{n}r{r}")
                      for r in range(r_lo, r_hi)}
                nsl = slice(n * 512, (n + 1) * 512)
                for k in range(K_TILES):
                    last = (k == K_TILES - 1)
                    for r in range(r_lo, r_hi):
                        col0 = half * R_HALF + r * 128
                        nc.tensor.matmul(
                            ps[r][:],
                            qxt_sb[k][:, col0:col0 + 128],
                            qwt_sb[k][:, nsl],
                            start=(k == 0), stop=last)
                        if last:
                            ot = ostage.tile([128, 512], BF16, tag="ot")
                            nc.vector.scalar_tensor_tensor(
                                ot[:], ps[r][:], sd, bias_full[:, nsl],
                                op0=mybir.AluOpType.mult,
                                op1=mybir.AluOpType.add)
                            row0 = half * R_HALF + r * 128
                            nc.sync.dma_start(
                                out_ext[row0:row0 + 128, nsl], ot[:])

            mm_block(0, 0, 0, 8)
            mm_block(0, 1, 0, 8)

            # ---- quantize x half1: emitted after the (0,*) blocks so its
            #      ACT/DVE ops queue behind their dequants, never ahead ----
            for k in range(K_TILES):
                quant(xt_sb[k][:, h1], qxt_sb[k][:, h1], sa, R_HALF)

            mm_block(1, 0, 0, 8)

            # ---- final block runs k-INNER per bank: every bank's
            #      dequant+store overlaps the next bank's matmuls, so the
            #      post-last-matmul tail is a single STT + store ----
            ps_f = {r: psum_pool.tile([128, 512], F32, tag="ps",
                                      name=f"ps_f{r}") for r in range(7)}
            for r in range(7):
                col0 = R_HALF + r * 128
                for k in range(K_TILES):
                    nc.tensor.matmul(
                        ps_f[r][:],
                        qxt_sb[k][:, col0:col0 + 128],
                        qwt_sb[k][:, n1],
                        start=(k == 0), stop=(k == K_TILES - 1))
                ot = ostage.tile([128, 512], BF16, tag="ot")
                nc.vector.scalar_tensor_tensor(
                    ot[:], ps_f[r][:], sd, bias_full[:, n1],
                    op0=mybir.AluOpType.mult, op1=mybir.AluOpType.add)
                nc.sync.dma_start(
                    out_ext[R_HALF + r * 128:R_HALF + (r + 1) * 128, n1],
                    ot[:])

            # very last bank as two 256-wide accumulation groups: the
            # first group's dequant starts one matmul before the end,
            # so the final tail is one [128,256] STT + store
            col0 = R_HALF + 7 * 128
            rows = slice(col0, col0 + 128)
            ps_a = psum_pool.tile([128, 256], F32, tag="ps", name="ps_f7a")
            ps_b = psum_pool.tile([128, 256], F32, tag="ps", name="ps_f7b")
            for k in range(K_TILES):
                nc.tensor.matmul(
                    ps_a[:], qxt_sb[k][:, col0:col0 + 128],
                    qwt_sb[k][:, 512:768],
                    start=(k == 0), stop=(k == K_TILES - 1))
                nc.tensor.matmul(
                    ps_b[:], qxt_sb[k][:, col0:col0 + 128],
                    qwt_sb[k][:, 768:1024],
                    start=(k == 0), stop=(k == K_TILES - 1))
            for half, ps_h in ((0, ps_a), (1, ps_b)):
                csl = slice(512 + half * 256, 768 + half * 256)
                ot = ostage.tile([128, 256], BF16, tag="ot")
                nc.vector.scalar_tensor_tensor(
                    ot[:], ps_h[:], sd, bias_full[:, csl],
                    op0=mybir.AluOpType.mult, op1=mybir.AluOpType.add)
                nc.sync.dma_start(out_ext[rows, csl], ot[:])

    nc.finalize()
    return nc


_NC_CACHE = None


def _get_nc():
    global _NC_CACHE
    if _NC_CACHE is None:
        _NC_CACHE = build_nc()
    return _NC_CACHE


def _scale(t):
    """f32 per-tensor scale exactly as the reference computes it."""
    t_max = np.float32(max(abs(float(t.min())), abs(float(t.max()))))
    if t_max == 0.0:
        t_max = np.float32(1.0)
    return np.float32(MAXV) / t_max, t_max


def make_in_maps(x, weight, bias):
    assert x.shape == (N_ROWS, SIZE_IN) and x.dtype == np.float32
    sa, _ = _scale(x)
    sw, _ = _scale(weight)
    sd = np.float32(1.0) / (sa * sw)
    consts = np.empty((128, 4), dtype=np.float32)
    consts[:, 0] = sa
    consts[:, 1] = sw
    consts[:, 2] = sd
    consts[:, 3] = np.float32(MAGIC)
    wt = np.ascontiguousarray(weight.T.astype(np.float16))
    b2 = np.ascontiguousarray(bias.reshape(1, SIZE_OUT))
    xt = x.T.astype(np.float16)
    in_maps = []
    for c in range(N_CORES):
        shard = np.ascontiguousarray(
            xt[:, c * ROWS_PER_CORE:(c + 1) * ROWS_PER_CORE])
        in_maps.append({"xt": shard, "wt": wt, "bias": b2, "consts": consts})
    return in_maps


def gather_out(res):
    return np.concatenate(
        [np.asarray(res.results[c]["out"]).astype(np.float32)
         for c in range(N_CORES)], axis=0)


def kernel(x, weight, bias):
    nc = _get_nc()
    in_maps = make_in_maps(x, weight, bias)
    res = run_bass_kernel_spmd(nc, in_maps, core_ids=list(range(N_CORES)))
    return gather_out(res)
